# revision 49
# baseline (speedup 1.0000x reference)
"""Trainium2 Bass kernel for KG-enhanced embedding model (gnn_message_passing).

Computes, for full inputs:
    inputs_embeds = word_embedding[input_ids]                       # [B,S,H] gather
    h   = relu(entity_embeddings @ W1 + b1)                         # [B,E,MLP_HID]
    ent = h @ W2 + b2                                               # [B,E,H]
    out = inputs_embeds + einsum('bes,beh->bsh', entity_mask, ent)  # masked scatter-add

Sharding: data-parallel over batch B=32 -> 4 examples per core on 8 cores.
Weights and the vocab table are replicated; the gather reads only the rows
each core needs via indirect DMA (16 x 128-row gathers per core).

Matmuls run in float32r (TF32-like: 11 explicit mantissa bits, fp32
accumulate) which streams the PE at 1 cycle/row for N>=256, ~4x faster
than fp32. The mask is 0/1 (exact); weights/activations are pre-rounded to
the f32r grid (round-to-nearest-even at 11 bits) so device matmuls are
deterministic. End-to-end absmax relative error ~1e-4.

Shapes (hardcoded): V=30522, H=768, B=32, S=512, E=8, KG=100, MH=1000.
"""

import os
import numpy as np
from contextlib import ExitStack

V, H = 30522, 768
B, S, E = 32, 512, 8
KG, MH = 100, 1000
NCORES = 8
BPC = B // NCORES              # examples per core = 4
TOK = BPC * S                  # tokens per core = 2048
NCH = TOK // 128               # 128-token chunks per core = 16
KCH = 8                        # K chunks of 128 for the 1000-dim contraction
NE = BPC * E                   # entities per core = 32

_PROGRAM = None


def _maybe_enable_profiling():
    """Optional NTFF profiling (KERNEL_PROFILE=1): shim antenv.axon_hooks."""
    if os.environ.get("KERNEL_PROFILE") != "1":
        return False
    import sys, types
    try:
        from antenv.axon_hooks import get_axon_ntff_profile_hook  # noqa: F401
        return True
    except ImportError:
        pass
    try:
        from trn_agent_boot.trn_boot import _ntff_profile_via_ctypes
        import antenv
        hook = _ntff_profile_via_ctypes("/opt/axon/libaxon_pjrt.so")
        m = types.ModuleType("antenv.axon_hooks")
        m.get_axon_ntff_profile_hook = lambda: hook
        m.set_axon_ntff_profile_hook = lambda h: None
        sys.modules["antenv.axon_hooks"] = m
        antenv.axon_hooks = m
        return True
    except Exception:
        return False


def _build_program():
    import concourse.bacc as bacc
    import concourse.tile as tile
    from concourse import bass, mybir

    f32 = mybir.dt.float32
    f32r = mybir.dt.float32r
    i32 = mybir.dt.int32
    RELU = mybir.ActivationFunctionType.Relu

    nc = bacc.Bacc("TRN2", target_bir_lowering=False, debug=False)

    ids_ap = nc.dram_tensor("idsT", [128, NCH], i32, kind="ExternalInput").ap()
    we_ap = nc.dram_tensor("we", [V, H], f32, kind="ExternalInput").ap()
    # w1ee packs w1r [KG, MH] and eeTr [KG, NE] side by side (one DMA)
    w1ee_ap = nc.dram_tensor("w1ee", [KG, MH + NE], f32r, kind="ExternalInput").ap()
    b1c_ap = nc.dram_tensor("b1colT", [128, KCH], f32, kind="ExternalInput").ap()
    w2_ap = nc.dram_tensor("w2pr", [128, KCH * H], f32r, kind="ExternalInput").ap()
    # b2o packs b2r [1, H] and ones [1, NE]
    b2o_ap = nc.dram_tensor("b2o", [1, H + NE], f32r, kind="ExternalInput").ap()
    maskT_ap = nc.dram_tensor("maskTr", [NE, TOK], f32r, kind="ExternalInput").ap()
    out_ap = nc.dram_tensor("out", [TOK, H], f32, kind="ExternalOutput").ap()

    with tile.TileContext(nc) as tc, ExitStack() as ctx:
        const = ctx.enter_context(tc.tile_pool(name="const", bufs=1))
        psA = ctx.enter_context(tc.tile_pool(name="psA", bufs=2, space="PSUM"))
        psB = ctx.enter_context(tc.tile_pool(name="psB", bufs=1, space="PSUM"))
        psC = ctx.enter_context(tc.tile_pool(name="psC", bufs=2, space="PSUM"))
        gpool = ctx.enter_context(tc.tile_pool(name="gath", bufs=NCH))
        opool = ctx.enter_context(tc.tile_pool(name="outp", bufs=6))

        # ---- PE warmup: dummy matmuls during the DMA-idle startup so the
        # HAM clock-gate ramps to 8/8 before the real MLP matmuls arrive.
        bf16 = mybir.dt.bfloat16
        wsc = const.tile([128, 512], bf16)
        nc.vector.memset(wsc[:], 0.0)
        # warmup psum shares psA's "ps" tag (both [128,512] f32 and the mm1
        # [128,32] psum fit one bank), so no extra PSUM banks are used
        for _ in range(10):
            wps = psA.tile([128, 512], f32, tag="ps")
            nc.tensor.matmul(
                out=wps[:], lhsT=wsc[:, :128], rhs=wsc[:], start=True, stop=True
            )

        # ---- loads: ids on scalar queue (gathers start immediately), packed
        # small weights first on sync so mm1/mm2 are never starved, then w2,
        # then gathers (SWDGE queue, shares SDMA bandwidth with everything)
        ids_sb = const.tile([128, NCH], i32)
        nc.scalar.dma_start(ids_sb[:], ids_ap[:])
        w1ee_sb = const.tile([KG, MH + NE], f32r)
        nc.sync.dma_start(w1ee_sb[:], w1ee_ap[:])
        w1_sb = w1ee_sb[:, :MH]
        eeT_sb = w1ee_sb[:, MH : MH + NE]
        b1_col = const.tile([128, KCH], f32)
        nc.sync.dma_start(b1_col[:], b1c_ap[:])
        b2o_sb = const.tile([1, H + NE], f32r)
        nc.sync.dma_start(b2o_sb[:], b2o_ap[:])
        b2_sb = b2o_sb[:, :H]
        ones1 = b2o_sb[:, H : H + NE]
        maskT_sb = const.tile([NE, TOK], f32r)
        nc.scalar.dma_start(maskT_sb[:], maskT_ap[:])
        # w2 split into quarters (separate tiles: slice-writes into one tile
        # would serialize on DMA completion) so mm2 k-chunks start as soon as
        # their quarter lands instead of waiting for the full 3.1MB
        KQ = KCH // 4  # k-chunks per quarter
        w2_q = []
        for q in range(4):
            wq = const.tile([128, KQ * H], f32r, tag=f"w2q{q}")
            nc.sync.dma_start(wq[:], w2_ap[:, q * KQ * H : (q + 1) * KQ * H])
            w2_q.append(wq)
        gts = []
        for g in range(NCH):
            gt = gpool.tile([128, H], f32)
            nc.gpsimd.indirect_dma_start(
                out=gt[:],
                out_offset=None,
                in_=we_ap[:],
                in_offset=bass.IndirectOffsetOnAxis(ap=ids_sb[:, g : g + 1], axis=0),
            )
            gts.append(gt)

        # ---- MLP stage 1: hT[k*128+p, e] = relu(W1.T @ ee.T + b1) ----------
        # b1 is added via the activation bias port (per-partition scalar).
        hT = const.tile([128, KCH, NE], f32r)
        hF = const.tile([128, KCH, NE], f32)
        # zero the tail of the last K-chunk (w2pr rows past MH are zero, but
        # garbage SBUF could be NaN: NaN*0=NaN). Memset can't write f32r;
        # round-copy from an f32 zeros tile instead.
        zf = const.tile([32, NE], f32)
        nc.vector.memset(zf[:], 0.0)
        nc.vector.tensor_copy(out=hT[96:128, KCH - 1, :], in_=zf[:])
        for k in range(KCH):
            mw = 128 if k < KCH - 1 else MH - 128 * (KCH - 1)  # 104 in last
            ps = psA.tile([128, NE], f32, tag="ps")
            nc.tensor.matmul(
                out=ps[:mw, :],
                lhsT=w1_sb[:, k * 128 : k * 128 + mw],
                rhs=eeT_sb[:],
                start=True,
                stop=True,
            )
            # h = relu(ps + b1) in f32, then round to the f32r grid
            nc.scalar.activation(
                out=hF[:mw, k, :],
                in_=ps[:mw, :],
                func=RELU,
                bias=b1_col[:mw, k : k + 1],
            )
            nc.vector.tensor_copy(out=hT[:mw, k, :], in_=hF[:mw, k, :])

        # ---- MLP stage 2: ent = hT.T @ W2 + b2 ------------------------------
        # b2 enters the PSUM accumulation as a K=1 matmul of ones.T @ b2.
        # entr held as one tile per n-group so n0 scatters can start while
        # the n1 accumulation is still streaming
        entp = psB.tile([NE, H], f32)
        entrs = {}
        for n0, n1 in ((0, 512), (512, H)):
            nc.tensor.matmul(
                out=entp[:, n0:n1],
                lhsT=ones1[:],
                rhs=b2_sb[:, n0:n1],
                start=True,
                stop=False,
            )
            for k in range(KCH):
                wq = w2_q[k // KQ]
                nc.tensor.matmul(
                    out=entp[:, n0:n1],
                    lhsT=hT[:, k, :],
                    rhs=wq[:, (k % KQ) * H + n0 : (k % KQ) * H + n1],
                    start=False,
                    stop=(k == KCH - 1),
                )
            er = const.tile([NE, n1 - n0], f32r, tag=f"entr{n0}")
            nc.vector.tensor_copy(out=er[:], in_=entp[:, n0:n1])
            entrs[n0] = er

        # ---- main loop: scatter-matmul, add, store -------------------------
        for g in range(NCH):
            gt = gts[g]
            sc = psC.tile([128, H], f32)
            for n0, n1 in ((0, 512), (512, H)):
                nc.tensor.matmul(
                    out=sc[:, n0:n1],
                    lhsT=maskT_sb[:, g * 128 : (g + 1) * 128],
                    rhs=entrs[n0][:],
                    start=True,
                    stop=True,
                )
            ot = opool.tile([128, H], f32)
            nc.vector.tensor_add(ot[:], gt[:], sc[:])
            st_eng = nc.sync if g % 2 == 0 else nc.scalar
            st_eng.dma_start(out_ap[g * 128 : (g + 1) * 128, :], ot[:])

    nc.compile()
    return nc


def _get_program():
    global _PROGRAM
    if _PROGRAM is None:
        _PROGRAM = _build_program()
    return _PROGRAM


def _round_f32r(x):
    """Round f32 to the f32r (TF32-like) grid: 11 explicit mantissa bits, RNE."""
    x = np.ascontiguousarray(x, dtype=np.float32)
    xi = x.view(np.uint32).astype(np.uint64)
    shift = np.uint64(23 - 11)
    add = np.uint64(1) << np.uint64(23 - 11 - 1)
    xi2 = ((xi + add) >> shift) << shift
    return np.ascontiguousarray(xi2.astype(np.uint32).view(np.float32))


def _prep_shards(inputs):
    ids = np.ascontiguousarray(np.asarray(inputs["input_ids"]).astype(np.int32))
    ee = np.asarray(inputs["entity_embeddings"], dtype=np.float32)
    mask = np.asarray(inputs["entity_mask"], dtype=np.float32)
    we = np.ascontiguousarray(np.asarray(inputs["word_embedding"], dtype=np.float32))
    W1 = np.asarray(inputs["W1"], dtype=np.float32)
    b1 = np.asarray(inputs["b1"], dtype=np.float32)
    W2 = np.asarray(inputs["W2"], dtype=np.float32)
    b2 = np.asarray(inputs["b2"], dtype=np.float32)

    w1r = _round_f32r(W1)  # [KG, MH]
    w2_pad = np.concatenate([W2, np.zeros((KCH * 128 - MH, H), np.float32)], 0)
    w2p = w2_pad.reshape(KCH, 128, H).transpose(1, 0, 2).reshape(128, KCH * H)
    w2pr = _round_f32r(w2p)
    b2o = np.concatenate(
        [_round_f32r(b2[None, :]), np.ones((1, NE), np.float32)], 1
    )  # [1, H+NE]
    b1pad = np.concatenate([b1, np.zeros(KCH * 128 - MH, np.float32)])
    b1colT = np.ascontiguousarray(b1pad.reshape(KCH, 128).T)  # [128, KCH]

    in_maps = []
    for i in range(NCORES):
        sl = slice(BPC * i, BPC * (i + 1))
        ids_i = ids[sl].reshape(-1)  # [TOK]
        idsT = np.ascontiguousarray(ids_i.reshape(NCH, 128).T)  # [128, NCH]
        eeTr = _round_f32r(ee[sl].reshape(NE, KG).T)  # [KG, NE]
        # block-diagonal [NE, TOK] mask: row b*E+e covers example b's columns
        maskT = np.zeros((NE, TOK), np.float32)
        for b in range(BPC):
            maskT[b * E : (b + 1) * E, b * S : (b + 1) * S] = mask[BPC * i + b]
        in_maps.append(
            {
                "idsT": idsT,
                "we": we,
                "w1ee": np.ascontiguousarray(np.concatenate([w1r, eeTr], 1)),
                "b1colT": b1colT,
                "w2pr": w2pr,
                "b2o": b2o,
                "maskTr": maskT,  # 0/1 values: exact on the f32r grid
            }
        )
    return in_maps


def kernel(**inputs) -> np.ndarray:
    from concourse.bass_utils import run_bass_kernel_spmd

    trace = _maybe_enable_profiling()
    nc = _get_program()
    in_maps = _prep_shards(inputs)
    res = run_bass_kernel_spmd(
        nc, in_maps, core_ids=list(range(NCORES)), trace=trace
    )
    if trace and res.exec_time_ns is not None:
        print(f"HW exec time: {res.exec_time_ns} ns")
    out = np.concatenate(
        [res.results[i]["out"].reshape(BPC, S, H) for i in range(NCORES)], 0
    )
    return out


if __name__ == "__main__":
    rng = np.random.default_rng(0)
    inputs = {
        "input_ids": rng.integers(0, V, (B, S)).astype(np.int32),
        "entity_embeddings": rng.standard_normal((B, E, KG), dtype=np.float32),
        "entity_mask": (rng.random((B, E, S)) < 0.02).astype(np.float32),
        "word_embedding": rng.standard_normal((V, H), dtype=np.float32) * 0.02,
        "W1": rng.standard_normal((KG, MH), dtype=np.float32) * 0.02,
        "b1": np.zeros(MH, np.float32),
        "W2": rng.standard_normal((MH, H), dtype=np.float32) * 0.02,
        "b2": np.zeros(H, np.float32),
    }
    out = kernel(**inputs)
    ref = inputs["word_embedding"][inputs["input_ids"]] + np.einsum(
        "bes,beh->bsh",
        inputs["entity_mask"],
        np.maximum(
            inputs["entity_embeddings"] @ inputs["W1"] + inputs["b1"], 0.0
        )
        @ inputs["W2"]
        + inputs["b2"],
    )
    err = np.abs(out - ref).max() / max(np.abs(ref).max(), 1e-12)
    print("self-check rel err:", err)


# revision 52
# speedup vs baseline: 1.0306x; 1.0306x over previous
"""Trainium2 Bass kernel for KG-enhanced embedding model (gnn_message_passing).

Computes, for full inputs:
    inputs_embeds = word_embedding[input_ids]                       # [B,S,H] gather
    h   = relu(entity_embeddings @ W1 + b1)                         # [B,E,MLP_HID]
    ent = h @ W2 + b2                                               # [B,E,H]
    out = inputs_embeds + einsum('bes,beh->bsh', entity_mask, ent)  # masked scatter-add

Sharding: data-parallel over batch B=32 -> 4 examples per core on 8 cores.
Weights and the vocab table are replicated; the gather reads only the rows
each core needs via indirect DMA (16 x 128-row gathers per core).

Matmuls run in float32r (TF32-like: 11 explicit mantissa bits, fp32
accumulate) which streams the PE at 1 cycle/row for N>=256, ~4x faster
than fp32. The mask is 0/1 (exact); weights/activations are pre-rounded to
the f32r grid (round-to-nearest-even at 11 bits) so device matmuls are
deterministic. End-to-end absmax relative error ~1e-4.

Shapes (hardcoded): V=30522, H=768, B=32, S=512, E=8, KG=100, MH=1000.
"""

import os
import numpy as np
from contextlib import ExitStack

V, H = 30522, 768
B, S, E = 32, 512, 8
KG, MH = 100, 1000
NCORES = 8
BPC = B // NCORES              # examples per core = 4
TOK = BPC * S                  # tokens per core = 2048
NCH = TOK // 128               # 128-token chunks per core = 16
KCH = 8                        # K chunks of 128 for the 1000-dim contraction
NE = BPC * E                   # entities per core = 32

_PROGRAM = None


def _maybe_enable_profiling():
    """Optional NTFF profiling (KERNEL_PROFILE=1): shim antenv.axon_hooks."""
    if os.environ.get("KERNEL_PROFILE") != "1":
        return False
    import sys, types
    try:
        from antenv.axon_hooks import get_axon_ntff_profile_hook  # noqa: F401
        return True
    except ImportError:
        pass
    try:
        from trn_agent_boot.trn_boot import _ntff_profile_via_ctypes
        import antenv
        hook = _ntff_profile_via_ctypes("/opt/axon/libaxon_pjrt.so")
        m = types.ModuleType("antenv.axon_hooks")
        m.get_axon_ntff_profile_hook = lambda: hook
        m.set_axon_ntff_profile_hook = lambda h: None
        sys.modules["antenv.axon_hooks"] = m
        antenv.axon_hooks = m
        return True
    except Exception:
        return False


def _build_program():
    import concourse.bacc as bacc
    import concourse.tile as tile
    from concourse import bass, mybir

    f32 = mybir.dt.float32
    f32r = mybir.dt.float32r
    i32 = mybir.dt.int32
    RELU = mybir.ActivationFunctionType.Relu

    nc = bacc.Bacc("TRN2", target_bir_lowering=False, debug=False)

    ids_ap = nc.dram_tensor("idsT", [128, NCH], i32, kind="ExternalInput").ap()
    we_ap = nc.dram_tensor("we", [V, H], f32, kind="ExternalInput").ap()
    # w1ee packs w1r [KG, MH] and eeTr [KG, NE] side by side (one DMA)
    w1ee_ap = nc.dram_tensor("w1ee", [KG, MH + NE], f32r, kind="ExternalInput").ap()
    b1c_ap = nc.dram_tensor("b1colT", [128, KCH], f32, kind="ExternalInput").ap()
    w2_ap = nc.dram_tensor("w2pr", [128, KCH * H], f32r, kind="ExternalInput").ap()
    # b2o packs b2r [1, H] and ones [1, NE]
    b2o_ap = nc.dram_tensor("b2o", [1, H + NE], f32r, kind="ExternalInput").ap()
    maskT_ap = nc.dram_tensor("maskTr", [NE, TOK], f32r, kind="ExternalInput").ap()
    out_ap = nc.dram_tensor("out", [TOK, H], f32, kind="ExternalOutput").ap()

    with tile.TileContext(nc) as tc, ExitStack() as ctx:
        const = ctx.enter_context(tc.tile_pool(name="const", bufs=1))
        psA = ctx.enter_context(tc.tile_pool(name="psA", bufs=2, space="PSUM"))
        psB = ctx.enter_context(tc.tile_pool(name="psB", bufs=1, space="PSUM"))
        psC = ctx.enter_context(tc.tile_pool(name="psC", bufs=2, space="PSUM"))
        gpool = ctx.enter_context(tc.tile_pool(name="gath", bufs=NCH))
        opool = ctx.enter_context(tc.tile_pool(name="outp", bufs=6))

        # ---- PE warmup: dummy matmuls during the DMA-idle startup so the
        # HAM clock-gate ramps to 8/8 before the real MLP matmuls arrive.
        bf16 = mybir.dt.bfloat16
        wsc = const.tile([128, 512], bf16)
        nc.vector.memset(wsc[:], 0.0)
        # warmup psum shares psA's "ps" tag (both [128,512] f32 and the mm1
        # [128,32] psum fit one bank), so no extra PSUM banks are used
        for _ in range(10):
            wps = psA.tile([128, 512], f32, tag="ps")
            nc.tensor.matmul(
                out=wps[:], lhsT=wsc[:, :128], rhs=wsc[:], start=True, stop=True
            )

        # ---- loads: ids on scalar queue (gathers start immediately), packed
        # small weights first on sync so mm1/mm2 are never starved, then w2,
        # then gathers (SWDGE queue, shares SDMA bandwidth with everything)
        ids_sb = const.tile([128, NCH], i32)
        nc.scalar.dma_start(ids_sb[:], ids_ap[:])
        w1ee_sb = const.tile([KG, MH + NE], f32r)
        nc.sync.dma_start(w1ee_sb[:], w1ee_ap[:])
        w1_sb = w1ee_sb[:, :MH]
        eeT_sb = w1ee_sb[:, MH : MH + NE]
        b1_col = const.tile([128, KCH], f32)
        nc.sync.dma_start(b1_col[:], b1c_ap[:])
        b2o_sb = const.tile([1, H + NE], f32r)
        nc.sync.dma_start(b2o_sb[:], b2o_ap[:])
        b2_sb = b2o_sb[:, :H]
        ones1 = b2o_sb[:, H : H + NE]
        maskT_sb = const.tile([NE, TOK], f32r)
        nc.scalar.dma_start(maskT_sb[:], maskT_ap[:])
        # w2 on the SWDGE (gpsimd) queue, issued BEFORE the gathers: FIFO
        # within the ring gives it strict priority over gather traffic, so
        # mm2 isn't starved; halves on separate tiles to pipeline mm2 entry
        KQ = KCH // 2
        w2_q = []
        for q in range(2):
            wq = const.tile([128, KQ * H], f32r, tag=f"w2q{q}")
            nc.gpsimd.dma_start(wq[:], w2_ap[:, q * KQ * H : (q + 1) * KQ * H])
            w2_q.append(wq)
        gts = []
        for g in range(NCH):
            gt = gpool.tile([128, H], f32)
            nc.gpsimd.indirect_dma_start(
                out=gt[:],
                out_offset=None,
                in_=we_ap[:],
                in_offset=bass.IndirectOffsetOnAxis(ap=ids_sb[:, g : g + 1], axis=0),
            )
            gts.append(gt)

        # ---- MLP stage 1: hT[k*128+p, e] = relu(W1.T @ ee.T + b1) ----------
        # b1 is added via the activation bias port (per-partition scalar).
        hT = const.tile([128, KCH, NE], f32r)
        hF = const.tile([128, KCH, NE], f32)
        # zero the tail of the last K-chunk (w2pr rows past MH are zero, but
        # garbage SBUF could be NaN: NaN*0=NaN). Memset can't write f32r;
        # round-copy from an f32 zeros tile instead.
        zf = const.tile([32, NE], f32)
        nc.vector.memset(zf[:], 0.0)
        nc.vector.tensor_copy(out=hT[96:128, KCH - 1, :], in_=zf[:])
        for k in range(KCH):
            mw = 128 if k < KCH - 1 else MH - 128 * (KCH - 1)  # 104 in last
            ps = psA.tile([128, NE], f32, tag="ps")
            nc.tensor.matmul(
                out=ps[:mw, :],
                lhsT=w1_sb[:, k * 128 : k * 128 + mw],
                rhs=eeT_sb[:],
                start=True,
                stop=True,
            )
            # h = relu(ps + b1) in f32, then round to the f32r grid
            nc.scalar.activation(
                out=hF[:mw, k, :],
                in_=ps[:mw, :],
                func=RELU,
                bias=b1_col[:mw, k : k + 1],
            )
            nc.vector.tensor_copy(out=hT[:mw, k, :], in_=hF[:mw, k, :])

        # ---- MLP stage 2: ent = hT.T @ W2 + b2 ------------------------------
        # b2 enters the PSUM accumulation as a K=1 matmul of ones.T @ b2.
        # entr held as one tile per n-group so n0 scatters can start while
        # the n1 accumulation is still streaming
        entp = psB.tile([NE, H], f32)
        entrs = {}
        for n0, n1 in ((0, 512), (512, H)):
            nc.tensor.matmul(
                out=entp[:, n0:n1],
                lhsT=ones1[:],
                rhs=b2_sb[:, n0:n1],
                start=True,
                stop=False,
            )
            for k in range(KCH):
                wq = w2_q[k // KQ]
                nc.tensor.matmul(
                    out=entp[:, n0:n1],
                    lhsT=hT[:, k, :],
                    rhs=wq[:, (k % KQ) * H + n0 : (k % KQ) * H + n1],
                    start=False,
                    stop=(k == KCH - 1),
                )
            er = const.tile([NE, n1 - n0], f32r, tag=f"entr{n0}")
            nc.vector.tensor_copy(out=er[:], in_=entp[:, n0:n1])
            entrs[n0] = er

        # ---- main loop: scatter-matmul, add, store -------------------------
        for g in range(NCH):
            gt = gts[g]
            sc = psC.tile([128, H], f32)
            for n0, n1 in ((0, 512), (512, H)):
                nc.tensor.matmul(
                    out=sc[:, n0:n1],
                    lhsT=maskT_sb[:, g * 128 : (g + 1) * 128],
                    rhs=entrs[n0][:],
                    start=True,
                    stop=True,
                )
            ot = opool.tile([128, H], f32)
            nc.vector.tensor_add(ot[:], gt[:], sc[:])
            st_eng = nc.sync if g % 2 == 0 else nc.scalar
            st_eng.dma_start(out_ap[g * 128 : (g + 1) * 128, :], ot[:])

    nc.compile()
    return nc


def _get_program():
    global _PROGRAM
    if _PROGRAM is None:
        _PROGRAM = _build_program()
    return _PROGRAM


def _round_f32r(x):
    """Round f32 to the f32r (TF32-like) grid: 11 explicit mantissa bits, RNE."""
    x = np.ascontiguousarray(x, dtype=np.float32)
    xi = x.view(np.uint32).astype(np.uint64)
    shift = np.uint64(23 - 11)
    add = np.uint64(1) << np.uint64(23 - 11 - 1)
    xi2 = ((xi + add) >> shift) << shift
    return np.ascontiguousarray(xi2.astype(np.uint32).view(np.float32))


def _prep_shards(inputs):
    ids = np.ascontiguousarray(np.asarray(inputs["input_ids"]).astype(np.int32))
    ee = np.asarray(inputs["entity_embeddings"], dtype=np.float32)
    mask = np.asarray(inputs["entity_mask"], dtype=np.float32)
    we = np.ascontiguousarray(np.asarray(inputs["word_embedding"], dtype=np.float32))
    W1 = np.asarray(inputs["W1"], dtype=np.float32)
    b1 = np.asarray(inputs["b1"], dtype=np.float32)
    W2 = np.asarray(inputs["W2"], dtype=np.float32)
    b2 = np.asarray(inputs["b2"], dtype=np.float32)

    w1r = _round_f32r(W1)  # [KG, MH]
    w2_pad = np.concatenate([W2, np.zeros((KCH * 128 - MH, H), np.float32)], 0)
    w2p = w2_pad.reshape(KCH, 128, H).transpose(1, 0, 2).reshape(128, KCH * H)
    w2pr = _round_f32r(w2p)
    b2o = np.concatenate(
        [_round_f32r(b2[None, :]), np.ones((1, NE), np.float32)], 1
    )  # [1, H+NE]
    b1pad = np.concatenate([b1, np.zeros(KCH * 128 - MH, np.float32)])
    b1colT = np.ascontiguousarray(b1pad.reshape(KCH, 128).T)  # [128, KCH]

    in_maps = []
    for i in range(NCORES):
        sl = slice(BPC * i, BPC * (i + 1))
        ids_i = ids[sl].reshape(-1)  # [TOK]
        idsT = np.ascontiguousarray(ids_i.reshape(NCH, 128).T)  # [128, NCH]
        eeTr = _round_f32r(ee[sl].reshape(NE, KG).T)  # [KG, NE]
        # block-diagonal [NE, TOK] mask: row b*E+e covers example b's columns
        maskT = np.zeros((NE, TOK), np.float32)
        for b in range(BPC):
            maskT[b * E : (b + 1) * E, b * S : (b + 1) * S] = mask[BPC * i + b]
        in_maps.append(
            {
                "idsT": idsT,
                "we": we,
                "w1ee": np.ascontiguousarray(np.concatenate([w1r, eeTr], 1)),
                "b1colT": b1colT,
                "w2pr": w2pr,
                "b2o": b2o,
                "maskTr": maskT,  # 0/1 values: exact on the f32r grid
            }
        )
    return in_maps


def kernel(**inputs) -> np.ndarray:
    from concourse.bass_utils import run_bass_kernel_spmd

    trace = _maybe_enable_profiling()
    nc = _get_program()
    in_maps = _prep_shards(inputs)
    res = run_bass_kernel_spmd(
        nc, in_maps, core_ids=list(range(NCORES)), trace=trace
    )
    if trace and res.exec_time_ns is not None:
        print(f"HW exec time: {res.exec_time_ns} ns")
    out = np.concatenate(
        [res.results[i]["out"].reshape(BPC, S, H) for i in range(NCORES)], 0
    )
    return out


if __name__ == "__main__":
    rng = np.random.default_rng(0)
    inputs = {
        "input_ids": rng.integers(0, V, (B, S)).astype(np.int32),
        "entity_embeddings": rng.standard_normal((B, E, KG), dtype=np.float32),
        "entity_mask": (rng.random((B, E, S)) < 0.02).astype(np.float32),
        "word_embedding": rng.standard_normal((V, H), dtype=np.float32) * 0.02,
        "W1": rng.standard_normal((KG, MH), dtype=np.float32) * 0.02,
        "b1": np.zeros(MH, np.float32),
        "W2": rng.standard_normal((MH, H), dtype=np.float32) * 0.02,
        "b2": np.zeros(H, np.float32),
    }
    out = kernel(**inputs)
    ref = inputs["word_embedding"][inputs["input_ids"]] + np.einsum(
        "bes,beh->bsh",
        inputs["entity_mask"],
        np.maximum(
            inputs["entity_embeddings"] @ inputs["W1"] + inputs["b1"], 0.0
        )
        @ inputs["W2"]
        + inputs["b2"],
    )
    err = np.abs(out - ref).max() / max(np.abs(ref).max(), 1e-12)
    print("self-check rel err:", err)
